# revision 23
# baseline (speedup 1.0000x reference)
"""Trainium2 Bass kernel: Mistral-style GQA attention with sliding-window mask.

Problem: hidden [1,2048,4096] -> Wq/Wk/Wv projections (32 q heads, 8 kv heads,
head_dim 128) -> RoPE -> sliding-window (1024) causal attention -> Wo.

Sharding: tensor-parallel over heads on 8 NeuronCores. Core i owns KV head i
and query heads 4i..4i+3 (Wq/Wk/Wv row-sharded, Wo column-sharded). Each core
computes partial_i = attn_heads_i @ Wo_i^T in HBM (bf16); host sums the 8
partials (the TP all-reduce) to produce the full output.

All HBM inputs are pre-cast to bf16 on host, so SBUF tiles are DMA'd directly
with no staging/cast ops. Projection PSUM accumulators are evicted to SBUF
bf16 with single wide copies immediately after the last matmul (freeing PSUM
banks for the attention stage early); RoPE then runs out of SBUF on DVE at
2x bf16 rate, ordered q0,q1 first so the first score matmuls un-block.
Attention is block-sparse over 512-query chunks with the previous chunk's Wo
matmul groups interleaved (10 up-front) to keep PE fed during rope/exp waits.
DMA is spread over four queues (ht / weights / stores / transposes).
"""

import sys

for _p in ("/opt/trn_rl_repo", "/root/.axon_site/_ro/trn_rl_repo"):
    if _p not in sys.path:
        sys.path.insert(0, _p)

import numpy as np
import ml_dtypes

import concourse.bass as bass  # noqa: F401  (registers engine classes)
import concourse.mybir as mybir
import concourse.tile as tile
from concourse import bacc
from concourse.bass_utils import run_bass_kernel_spmd

S = 2048
HID = 4096
D = 128
NQH = 4          # query heads per core
NCORES = 8
SC = 512         # seq chunk
NCH = S // SC
KT = HID // 128  # contraction tiles
WINDOW = 1024
ROPE_BASE = 10000.0
SCALE = 1.0 / float(np.sqrt(D))

F32 = mybir.dt.float32
BF16 = mybir.dt.bfloat16
MULT = mybir.AluOpType.mult
ADD = mybir.AluOpType.add
EXP = mybir.ActivationFunctionType.Exp

# ptb slot layout: slot sl = kb - 4c + 8 for key-block kb in chunk c.
def _slot_region(sl):
    lo = 128 * (sl - 8) if sl >= 8 else 0
    hi = 512 if sl >= 3 else 128 * (sl + 1)
    return lo, hi


def _program(tc, dr, out, niter=1, fused=True):
    nc = tc.nc
    ht, wqt, wkt, wvt, wot = dr["ht"], dr["wqt"], dr["wkt"], dr["wvt"], dr["wot"]
    ctab, stab, mcaus, mwin = dr["ctab"], dr["stab"], dr["mcaus"], dr["mwin"]

    def _copy(eng, out_ap, in_ap):
        if eng is nc.scalar:
            eng.copy(out_ap, in_ap)
        else:
            eng.tensor_copy(out_ap, in_ap)

    # psum-reading copy engines (gpsimd cannot touch PSUM)
    pcopy_engines = [nc.vector, nc.scalar, nc.vector, nc.vector]
    pcopy_idx = [0]

    def pcopy_rr(out_ap, in_ap):
        _copy(pcopy_engines[pcopy_idx[0] % 4], out_ap, in_ap)
        pcopy_idx[0] += 1

    from contextlib import ExitStack
    if True:
        with ExitStack() as ctx:
            pw = ctx.enter_context(tc.tile_pool(name="persist", bufs=1))
            prt = ctx.enter_context(tc.tile_pool(name="ropet", bufs=2))
            pev = ctx.enter_context(tc.tile_pool(name="evt", bufs=6))

            wqb = pw.tile([128, KT * 512], BF16, name="wqb")
            wkb = pw.tile([128, KT * 128], BF16, name="wkb")
            wvb = pw.tile([128, KT * 128], BF16, name="wvb")
            wob = pw.tile([128, NQH * HID], BF16, name="wob")
            qtb = [pw.tile([128, S], BF16, name=f"qtb{h}") for h in range(NQH)]
            ktb = pw.tile([128, S], BF16, name="ktb")
            vnat = pw.tile([128, S], BF16, name="vnat")
            cs_t = pw.tile([128, S], BF16, name="cs_t")
            sn_t = pw.tile([128, S], BF16, name="sn_t")
            mc_t = pw.tile([128, 128], BF16, name="mc_t")
            mw_t = pw.tile([128, 128], BF16, name="mw_t")
            ones_t = pw.tile([128, 1], BF16, name="ones_t")

            # one-time loads (scalar/ACT HWDGE queue): batched strided weight
            # DMAs keep the trigger count low; Q/K/V group 0 first so the
            # first projection matmuls un-block earliest, consts last.
            for g in range(4):
                nc.scalar.dma_start(
                    wqb[:, 4096 * g:4096 * (g + 1)].rearrange(
                        "p (k d) -> p k d", d=512),
                    wqt[1024 * g:1024 * (g + 1), :].rearrange(
                        "(k p) d -> p k d", p=128))
                nc.scalar.dma_start(
                    wkb[:, 1024 * g:1024 * (g + 1)].rearrange(
                        "p (k d) -> p k d", d=128),
                    wkt[1024 * g:1024 * (g + 1), :].rearrange(
                        "(k p) d -> p k d", p=128))
                nc.scalar.dma_start(
                    wvb[:, 1024 * g:1024 * (g + 1)].rearrange(
                        "p (k d) -> p k d", d=128),
                    wvt[1024 * g:1024 * (g + 1), :].rearrange(
                        "(k p) d -> p k d", p=128))
            # wot rows (h p) map straight onto wob's [p, (h d)] layout.
            for g in range(2):
                nc.scalar.dma_start(
                    wob[:, 2 * HID * g:2 * HID * (g + 1)].rearrange(
                        "p (h d) -> p h d", d=HID),
                    wot[256 * g:256 * (g + 1), :].rearrange(
                        "(h p) d -> p h d", p=128))
            nc.scalar.dma_start(cs_t[:], ctab[:])
            nc.scalar.dma_start(sn_t[:], stab[:])
            nc.scalar.dma_start(mc_t[:], mcaus[:])
            nc.scalar.dma_start(mw_t[:], mwin[:])
            nc.gpsimd.memset(ones_t[:], 1.0)

            def _rope(dst, ev, c):
                """dst[bf16 [128,512] slice] = rope(ev [sbuf bf16 [128,512]]).

                cs_t is cos duplicated across both halves; sn_t is sign-baked
                sin: rows 0:64 = -sin, rows 64:128 = +sin, so
                out = q*cos + rot(q)*sn with rot a plain half-swap.
                The half-swap runs as two gpsimd copies (TensorTensor operands
                must share a start partition on HW); the three multiplies/adds
                are full-height all-bf16 SBUF DVE ops (2x mode).
                """
                csl = cs_t[:, SC * c:SC * (c + 1)]
                snl = sn_t[:, SC * c:SC * (c + 1)]
                rot = prt.tile([128, 512], BF16, tag="rot", name="rot")
                nc.sync.dma_start(rot[0:64, :], ev[64:128, :])
                nc.sync.dma_start(rot[64:128, :], ev[0:64, :])
                t1 = prt.tile([128, 512], BF16, tag="rt1", name="rt1")
                t2 = prt.tile([128, 512], BF16, tag="rt2", name="rt2")
                nc.vector.tensor_tensor(t1[:], ev[:], csl, MULT)
                nc.vector.tensor_tensor(t2[:], rot[:], snl, MULT)
                nc.vector.tensor_tensor(dst, t1[:], t2[:], ADD)

            phb = ctx.enter_context(tc.tile_pool(name="htbp", bufs=10))
            ppt = ctx.enter_context(tc.tile_pool(name="ptp", bufs=7))
            pmisc = ctx.enter_context(tc.tile_pool(name="miscb", bufs=2))
            pat = ctx.enter_context(tc.tile_pool(name="atbp", bufs=16))
            posb = ctx.enter_context(tc.tile_pool(name="osbp", bufs=2))
            ppvf = ctx.enter_context(tc.tile_pool(name="pvfb", bufs=4))
            # persistent Wo PSUM pool: proj uses 6 banks, attn uses sc2+pv2+den2,
            # so 2 banks stay available for Wo groups throughout.
            pop = ctx.enter_context(tc.tile_pool(name="outps", bufs=2, space="PSUM"))

            # per-chunk pipeline: projections (+weight streaming on c==0/1),
            # then block-sparse attention, then Wo partial + output store.
            def proj_stage(c):
                with tc.tile_pool(name="projps", bufs=6, space="PSUM") as ppp:
                    ps6 = [ppp.tile([128, 512], F32, tag="proj", name=f"proj{c}_{i}")
                           for i in range(6)]
                    for k in range(KT):
                        hb = phb.tile([128, 512], BF16, tag="htb", name="hb")
                        nc.sync.dma_start(hb[:], ht[128 * k:128 * (k + 1),
                                                    SC * c:SC * (c + 1)])
                        first, last = k == 0, k == KT - 1
                        for h in range(NQH):
                            nc.tensor.matmul(ps6[h][:], wqb[:, 512 * k + 128 * h:512 * k + 128 * (h + 1)],
                                             hb[:], start=first, stop=last)
                        nc.tensor.matmul(ps6[4][:], wkb[:, 128 * k:128 * (k + 1)], hb[:],
                                         start=first, stop=last)
                        # V in natural [seq, dv] layout: hb-slice stationary,
                        # wvb moving -> no DMA transpose needed later.
                        # start=True clears has_written for the WHOLE bank, so
                        # only the very first sub-matmul may carry it; the
                        # other b4 regions auto-overwrite on first touch.
                        for b4 in range(4):
                            nc.tensor.matmul(ps6[5][:, 128 * b4:128 * (b4 + 1)],
                                             hb[:, 128 * b4:128 * (b4 + 1)],
                                             wvb[:, 128 * k:128 * (k + 1)],
                                             start=(first and b4 == 0), stop=last,
                                             skip_group_check=True)
                    # fast PSUM eviction (bf16): single wide copies free banks
                    # for the attention stage; rope then runs out of SBUF.
                    # q0/q1 first so pair-0 scores un-block earliest.
                    evs = [None] * 5
                    evs[0] = pev.tile([128, 512], BF16, tag="ev", name="ev0")
                    nc.scalar.copy(evs[0][:], ps6[0][:])
                    evs[1] = pev.tile([128, 512], BF16, tag="ev", name="ev1")
                    nc.scalar.copy(evs[1][:], ps6[1][:])
                    evs[2] = pev.tile([128, 512], BF16, tag="ev", name="ev2")
                    nc.vector.tensor_copy(evs[2][:], ps6[2][:])
                    evs[3] = pev.tile([128, 512], BF16, tag="ev", name="ev3")
                    nc.vector.tensor_copy(evs[3][:], ps6[3][:])
                    evs[4] = pev.tile([128, 512], BF16, tag="ev", name="ev4")
                    nc.scalar.copy(evs[4][:], ps6[4][:])
                    nc.scalar.copy(vnat[:, SC * c:SC * (c + 1)], ps6[5][:])
                _rope(qtb[0][:, SC * c:SC * (c + 1)], evs[0], c)
                _rope(qtb[1][:, SC * c:SC * (c + 1)], evs[1], c)
                _rope(ktb[:, SC * c:SC * (c + 1)], evs[4], c)
                while deferred:
                    deferred.pop(0)()
                return evs

            wo_ob = [None]

            def emit_wo_group(wc, wj, wn, watbs):
                """One Wo output tile [128q, 512hid] for chunk wc: 4 head-MMs,
                PSUM->SBUF copy (bf16). Groups of one query block share a wide
                ob tile; a single [128,4096] store fires on the last group."""
                po = pop.tile([128, 512], F32, tag="po", name="po")
                for h in range(NQH):
                    nc.tensor.matmul(po[:], watbs[h][:, 128 * wj:128 * (wj + 1)],
                                     wob[:, HID * h + 512 * wn:HID * h + 512 * (wn + 1)],
                                     start=(h == 0), stop=(h == NQH - 1))
                if wn == 0:
                    wo_ob[0] = posb.tile([128, HID], BF16, tag="osb", name="osb")
                ob = wo_ob[0]
                pcopy_rr(ob[:, 512 * wn:512 * (wn + 1)], po[:])
                if wn == 7:
                    # scalar queue: keeps wide stores off the ht-stream queue
                    nc.scalar.dma_start(out[SC * wc + 128 * wj:SC * wc + 128 * (wj + 1), :],
                                        ob[:])

            def attn_stage(c, prev, evs):
                # ---- attention for this chunk (past K/V only: sliding window),
                # with the PREVIOUS chunk's Wo matmul groups interleaved between
                # key-blocks so PE has independent work during rope/exp waits.
                wo_pending = []
                if prev is not None:
                    pc, patbs = prev
                    wo_pending = [(pc, j, n, patbs) for j in range(4) for n in range(8)]
                with tc.tile_pool(name="scps", bufs=3, space="PSUM") as psc, \
                     tc.tile_pool(name="pvps", bufs=2, space="PSUM") as ppv, \
                     tc.tile_pool(name="denps", bufs=1, space="PSUM") as pdn:
                    kbs = list(range(max(0, 4 * c - 8), 4 * c + 4))
                    first_kb, last_kb = kbs[0], kbs[-1]
                    # give PE independent work while DVE runs this chunk's rope
                    for _ in range(min(8, len(wo_pending))):
                        emit_wo_group(*wo_pending.pop(0))
                    atbs = []
                    for h0 in range(0, NQH, 2):
                        if h0 == 2:
                            # q2/q3 rope lands here so its DVE/DMA work queues
                            # behind pair-0's masks/copies, not ahead of them.
                            _rope(qtb[2][:, SC * c:SC * (c + 1)], evs[2], c)
                            _rope(qtb[3][:, SC * c:SC * (c + 1)], evs[3], c)
                        # process a PAIR of heads per key-block sweep: two
                        # independent score/exp chains per step keep ACT fed.
                        pvs = [ppv.tile([128, 512], F32, tag="pv", name="pv")
                               for _ in range(2)]
                        dpk = pdn.tile([2, 512], F32, tag="den", name="den")
                        dens = [dpk[0:1, :], dpk[1:2, :]]

                        def emit_pv(kb, pts):
                            # accumulate P@V and row-sums over exact visible slices.
                            sl = kb - 4 * c + 8
                            lo, hi = _slot_region(sl)
                            vsl = vnat[:, 128 * kb:128 * (kb + 1)]
                            for i in range(2):
                                nc.tensor.matmul(pvs[i][:, lo:hi], vsl, pts[i][:, lo:hi],
                                                 start=(kb == first_kb), stop=(kb == last_kb),
                                                 skip_group_check=True)
                                # dens share one bank: only the very first write
                                # carries start (it clears the WHOLE bank).
                                nc.tensor.matmul(dens[i][:, lo:hi], ones_t[:], pts[i][:, lo:hi],
                                                 start=(kb == first_kb and i == 0),
                                                 stop=(kb == last_kb),
                                                 skip_group_check=True)

                        prevkb = None
                        for kb in kbs:
                            sl = kb - 4 * c + 8
                            lo, hi = _slot_region(sl)
                            pts = []
                            for i in range(2):
                                sc = psc.tile([128, 512], F32, tag="sc", name="sc")
                                nc.tensor.matmul(sc[:, lo:hi], ktb[:, 128 * kb:128 * (kb + 1)],
                                                 qtb[h0 + i][:, SC * c + lo:SC * c + hi],
                                                 start=True, stop=True)
                                pt = ppt.tile([128, 512], BF16, tag="pt", name="pt")
                                nc.scalar.activation(pt[:, lo:hi], sc[:, lo:hi], EXP, scale=SCALE)
                                if sl <= 3:
                                    mofs = 128 * sl
                                    nc.gpsimd.tensor_tensor(pt[:, mofs:mofs + 128],
                                                            pt[:, mofs:mofs + 128], mw_t[:], MULT)
                                elif sl >= 8:
                                    mofs = 128 * (sl - 8)
                                    nc.gpsimd.tensor_tensor(pt[:, mofs:mofs + 128],
                                                            pt[:, mofs:mofs + 128], mc_t[:], MULT)
                                pts.append(pt)
                            if prevkb is not None:
                                emit_pv(*prevkb)
                            npop = 2 if (hi - lo) < 400 else 1
                            for _ in range(min(npop, len(wo_pending))):
                                emit_wo_group(*wo_pending.pop(0))
                            prevkb = (kb, pts)
                        emit_pv(*prevkb)
                        if wo_pending:
                            emit_wo_group(*wo_pending.pop(0))
                        last_pair = h0 == 2
                        for i in range(2):
                            # evict pv/den out of PSUM promptly (frees banks for
                            # the next proj stage), then normalize from SBUF.
                            # The LAST pair's broadcast+multiply are deferred
                            # into the next proj stage: their results are only
                            # needed a chunk later, and running them here puts
                            # them ahead of the next chunk's rope in the
                            # GP/DVE FIFOs (priority inversion).
                            pvf = ppvf.tile([128, 512], F32, tag="pvf", name="pvf")
                            _copy(nc.scalar if i == 0 else nc.vector, pvf[:], pvs[i][:])
                            dre2 = pmisc.tile([2, 512], F32, tag="denr", name="denr")
                            dre = dre2[i:i + 1, :]
                            nc.vector.reciprocal(dre, dens[i])
                            at = pat.tile([128, 512], BF16, tag="atb", name="atb")

                            def _norm(pvf=pvf, dre=dre, at=at):
                                dbc = pmisc.tile([128, 512], F32, tag="denb", name="denb")
                                nc.gpsimd.partition_broadcast(dbc[:], dre)
                                nc.vector.tensor_tensor(at[:], pvf[:], dbc[:], MULT)

                            if last_pair:
                                deferred.append(_norm)
                            else:
                                _norm()
                            atbs.append(at)
                    while wo_pending:
                        emit_wo_group(*wo_pending.pop(0))
                return atbs

            prev = None
            deferred = []
            for _it in range(niter):
                if fused:
                    for c in range(NCH):
                        evs = proj_stage(c)
                        prev = (c, attn_stage(c, prev, evs))
                else:
                    allevs = []
                    for c in range(NCH):
                        allevs.append(proj_stage(c))
                    for c in range(NCH):
                        prev = (c, attn_stage(c, prev, allevs[c]))
            # final chunk's Wo tail
            while deferred:
                deferred.pop(0)()
            pc, patbs = prev
            for j in range(4):
                for n in range(8):
                    emit_wo_group(pc, j, n, patbs)
            if "dbg_q0" in dr:
                nc.sync.dma_start(dr["dbg_q0"], qtb[0][:])
                nc.sync.dma_start(dr["dbg_k"], ktb[:])
                nc.sync.dma_start(dr["dbg_v"], vnat[:])
                for hh in range(4):
                    nc.sync.dma_start(dr[f"dbg_at{hh}"], patbs[hh][:])


_NC_CACHE = {}


def _build(niter=1, fused=True):
    import os
    fused = os.environ.get("KERNEL_FUSED", "1" if fused else "0") == "1"
    key = (niter, fused)
    if key in _NC_CACHE:
        return _NC_CACHE[key]
    nc = bacc.Bacc("TRN2", target_bir_lowering=False, debug=False,
                   enable_asserts=True, num_devices=NCORES)
    dr = {}

    def din(name, shape, dt=F32):
        dr[name] = nc.dram_tensor(name, shape, dt, kind="ExternalInput").ap()

    din("ht", [HID, S], BF16)
    din("wqt", [HID, NQH * D], BF16)
    din("wkt", [HID, D], BF16)
    din("wvt", [HID, D], BF16)
    din("wot", [NQH * D, HID], BF16)
    din("ctab", [128, S], BF16)
    din("stab", [128, S], BF16)
    din("mcaus", [128, 128], BF16)
    din("mwin", [128, 128], BF16)
    out = nc.dram_tensor("out", [S, HID], BF16, kind="ExternalOutput").ap()
    if os.environ.get("KERNEL_DEBUG") == "1":
        for nm, shp in [("dbg_q0", [128, S]), ("dbg_k", [128, S]), ("dbg_v", [128, S]),
                        ("dbg_at0", [128, 512]), ("dbg_at1", [128, 512]),
                        ("dbg_at2", [128, 512]), ("dbg_at3", [128, 512])]:
            dr[nm] = nc.dram_tensor(nm, shp, BF16, kind="ExternalOutput").ap()

    with tile.TileContext(nc) as tc:
        _program(tc, dr, out, niter, fused)
    nc.compile()
    _NC_CACHE[key] = nc
    return nc


def make_in_maps(inputs):
    hs = np.asarray(inputs["hidden_states"], dtype=np.float32)
    Wq = np.asarray(inputs["Wq"], dtype=np.float32)
    Wk = np.asarray(inputs["Wk"], dtype=np.float32)
    Wv = np.asarray(inputs["Wv"], dtype=np.float32)
    Wo = np.asarray(inputs["Wo"], dtype=np.float32)
    pos = np.asarray(inputs["position_ids"]).reshape(-1)

    assert hs.shape == (1, S, HID), hs.shape
    H = hs[0]
    BF = ml_dtypes.bfloat16
    HT = np.ascontiguousarray(H.T.astype(BF))

    # RoPE tables in [d%64, s] layout (bf16, mirroring the reference math)
    inv = (1.0 / (ROPE_BASE ** (np.arange(0, D, 2, dtype=np.float32) / D))).astype(np.float32)
    ang = pos.astype(np.float32)[None, :] * inv[:, None]          # [64, S]
    cos64 = np.cos(ang).astype(np.float32)
    sin64 = np.sin(ang).astype(np.float32)
    ctab = np.concatenate([cos64, cos64], axis=0).astype(BF)      # [128, S]
    stab = np.concatenate([-sin64, sin64], axis=0).astype(BF)     # sign-baked

    kk = np.arange(128)[:, None]
    qq = np.arange(128)[None, :]
    mcaus = (qq >= kk).astype(BF)   # causal diag block, [k,q]
    mwin = (qq < kk).astype(BF)     # window-edge block, [k,q]

    in_maps = []
    for i in range(NCORES):
        in_maps.append({
            "ht": HT,
            "wqt": np.ascontiguousarray(Wq[512 * i:512 * (i + 1), :].T.astype(BF)),
            "wkt": np.ascontiguousarray(Wk[128 * i:128 * (i + 1), :].T.astype(BF)),
            "wvt": np.ascontiguousarray(Wv[128 * i:128 * (i + 1), :].T.astype(BF)),
            "wot": np.ascontiguousarray(Wo[:, 512 * i:512 * (i + 1)].T.astype(BF)),
            "ctab": ctab,
            "stab": stab,
            "mcaus": mcaus,
            "mwin": mwin,
        })

    return in_maps


def kernel(**inputs):
    in_maps = make_in_maps(inputs)
    nc = _build()
    res = run_bass_kernel_spmd(nc, in_maps, core_ids=list(range(NCORES)))

    acc = np.zeros((S, HID), dtype=np.float64)
    for r in res.results:
        acc += r["out"].astype(np.float64)
    return acc.astype(np.float32).reshape(1, S, HID)


# revision 29
# speedup vs baseline: 1.4243x; 1.4243x over previous
"""Trainium2 Bass kernel: Mistral-style GQA attention with sliding-window mask.

Problem: hidden [1,2048,4096] -> Wq/Wk/Wv projections (32 q heads, 8 kv heads,
head_dim 128) -> RoPE -> sliding-window (1024) causal attention -> Wo.

Sharding: tensor-parallel over heads on 8 NeuronCores. Core i owns KV head i
and query heads 4i..4i+3 (Wq/Wk/Wv row-sharded, Wo column-sharded). Each core
computes partial_i = attn_heads_i @ Wo_i^T in HBM (bf16); host sums the 8
partials (the TP all-reduce) to produce the full output.

All HBM inputs are pre-cast to bf16 on host, so SBUF tiles are DMA'd directly
with no staging/cast ops. Projection PSUM accumulators are evicted to SBUF
bf16 with single wide copies immediately after the last matmul (freeing PSUM
banks for the attention stage early); RoPE then runs out of SBUF on DVE at
2x bf16 rate, ordered q0,q1 first so the first score matmuls un-block.
Attention is block-sparse over 512-query chunks with the previous chunk's Wo
matmul groups interleaved (10 up-front) to keep PE fed during rope/exp waits.
DMA is spread over four queues (ht / weights / stores / transposes).
"""

import sys

for _p in ("/opt/trn_rl_repo", "/root/.axon_site/_ro/trn_rl_repo"):
    if _p not in sys.path:
        sys.path.insert(0, _p)

import numpy as np
import ml_dtypes

import concourse.bass as bass  # noqa: F401  (registers engine classes)
import concourse.mybir as mybir
import concourse.tile as tile
from concourse import bacc
from concourse.bass_utils import run_bass_kernel_spmd

S = 2048
HID = 4096
D = 128
NQH = 4          # query heads per core
NCORES = 8
SC = 512         # seq chunk
NCH = S // SC
KT = HID // 128  # contraction tiles
WINDOW = 1024
ROPE_BASE = 10000.0
SCALE = 1.0 / float(np.sqrt(D))

F32 = mybir.dt.float32
BF16 = mybir.dt.bfloat16
MULT = mybir.AluOpType.mult
ADD = mybir.AluOpType.add
EXP = mybir.ActivationFunctionType.Exp

# ptb slot layout: slot sl = kb - 4c + 8 for key-block kb in chunk c.
def _slot_region(sl):
    lo = 128 * (sl - 8) if sl >= 8 else 0
    hi = 512 if sl >= 3 else 128 * (sl + 1)
    return lo, hi


def _program(tc, dr, out, niter=1, fused=True):
    nc = tc.nc
    ht, wqt, wkt, wvt, wot = dr["ht"], dr["wqt"], dr["wkt"], dr["wvt"], dr["wot"]
    ctab, stab, mcaus, mwin = dr["ctab"], dr["stab"], dr["mcaus"], dr["mwin"]

    def _copy(eng, out_ap, in_ap):
        if eng is nc.scalar:
            eng.copy(out_ap, in_ap)
        else:
            eng.tensor_copy(out_ap, in_ap)

    # psum-reading copy engines (gpsimd cannot touch PSUM)
    pcopy_engines = [nc.vector, nc.scalar, nc.vector, nc.vector]
    pcopy_idx = [0]

    def pcopy_rr(out_ap, in_ap):
        _copy(pcopy_engines[pcopy_idx[0] % 4], out_ap, in_ap)
        pcopy_idx[0] += 1

    from contextlib import ExitStack
    if True:
        with ExitStack() as ctx:
            pw = ctx.enter_context(tc.tile_pool(name="persist", bufs=1))
            prt = ctx.enter_context(tc.tile_pool(name="ropet", bufs=2))
            pev = ctx.enter_context(tc.tile_pool(name="evt", bufs=6))

            wqb = pw.tile([128, KT * 512], BF16, name="wqb")
            wkb = pw.tile([128, KT * 128], BF16, name="wkb")
            wvb = pw.tile([128, KT * 128], BF16, name="wvb")
            wob = pw.tile([128, NQH * HID], BF16, name="wob")
            qtb = [pw.tile([128, S], BF16, name=f"qtb{h}") for h in range(NQH)]
            ktb = pw.tile([128, S], BF16, name="ktb")
            vnat = pw.tile([128, S], BF16, name="vnat")
            cs_t = pw.tile([128, S], BF16, name="cs_t")
            sn_t = pw.tile([128, S], BF16, name="sn_t")
            mc_t = pw.tile([128, 128], BF16, name="mc_t")
            mw_t = pw.tile([128, 128], BF16, name="mw_t")
            ones_t = pw.tile([128, 1], BF16, name="ones_t")

            # one-time loads (scalar/ACT HWDGE queue): batched strided weight
            # DMAs keep the trigger count low; Q/K/V group 0 first so the
            # first projection matmuls un-block earliest, consts last.
            for g in range(4):
                nc.scalar.dma_start(
                    wqb[:, 4096 * g:4096 * (g + 1)].rearrange(
                        "p (k d) -> p k d", d=512),
                    wqt[1024 * g:1024 * (g + 1), :].rearrange(
                        "(k p) d -> p k d", p=128))
                nc.scalar.dma_start(
                    wkb[:, 1024 * g:1024 * (g + 1)].rearrange(
                        "p (k d) -> p k d", d=128),
                    wkt[1024 * g:1024 * (g + 1), :].rearrange(
                        "(k p) d -> p k d", p=128))
                nc.scalar.dma_start(
                    wvb[:, 1024 * g:1024 * (g + 1)].rearrange(
                        "p (k d) -> p k d", d=128),
                    wvt[1024 * g:1024 * (g + 1), :].rearrange(
                        "(k p) d -> p k d", p=128))
            # wot rows (h p) map straight onto wob's [p, (h d)] layout.
            for g in range(2):
                nc.scalar.dma_start(
                    wob[:, 2 * HID * g:2 * HID * (g + 1)].rearrange(
                        "p (h d) -> p h d", d=HID),
                    wot[256 * g:256 * (g + 1), :].rearrange(
                        "(h p) d -> p h d", p=128))
            nc.scalar.dma_start(cs_t[:], ctab[:])
            nc.scalar.dma_start(sn_t[:], stab[:])
            nc.scalar.dma_start(mc_t[:], mcaus[:])
            nc.scalar.dma_start(mw_t[:], mwin[:])
            nc.gpsimd.memset(ones_t[:], 1.0)

            def _rope(dst, ev, c):
                """dst[bf16 [128,512] slice] = rope(ev [sbuf bf16 [128,512]]).

                cs_t is cos duplicated across both halves; sn_t is sign-baked
                sin: rows 0:64 = -sin, rows 64:128 = +sin, so
                out = q*cos + rot(q)*sn with rot a plain half-swap.
                The half-swap runs as two gpsimd copies (TensorTensor operands
                must share a start partition on HW); the three multiplies/adds
                are full-height all-bf16 SBUF DVE ops (2x mode).
                """
                csl = cs_t[:, SC * c:SC * (c + 1)]
                snl = sn_t[:, SC * c:SC * (c + 1)]
                rot = prt.tile([128, 512], BF16, tag="rot", name="rot")
                nc.sync.dma_start(rot[0:64, :], ev[64:128, :])
                nc.sync.dma_start(rot[64:128, :], ev[0:64, :])
                t1 = prt.tile([128, 512], BF16, tag="rt1", name="rt1")
                t2 = prt.tile([128, 512], BF16, tag="rt2", name="rt2")
                nc.vector.tensor_tensor(t1[:], ev[:], csl, MULT)
                nc.vector.tensor_tensor(t2[:], rot[:], snl, MULT)
                nc.vector.tensor_tensor(dst, t1[:], t2[:], ADD)

            phb = ctx.enter_context(tc.tile_pool(name="htbp", bufs=10))
            ppt = ctx.enter_context(tc.tile_pool(name="ptp", bufs=7))
            pmisc = ctx.enter_context(tc.tile_pool(name="miscb", bufs=2))
            pat = ctx.enter_context(tc.tile_pool(name="atbp", bufs=16))
            posb = ctx.enter_context(tc.tile_pool(name="osbp", bufs=2))
            ppvf = ctx.enter_context(tc.tile_pool(name="pvfb", bufs=4))
            # persistent Wo PSUM pool: proj uses 6 banks, attn uses sc2+pv2+den2,
            # so 2 banks stay available for Wo groups throughout.
            pop = ctx.enter_context(tc.tile_pool(name="outps", bufs=2, space="PSUM"))

            # per-chunk pipeline: projections (+weight streaming on c==0/1),
            # then block-sparse attention, then Wo partial + output store.
            def proj_stage(c):
                with tc.tile_pool(name="projps", bufs=6, space="PSUM") as ppp:
                    ps6 = [ppp.tile([128, 512], F32, tag="proj", name=f"proj{c}_{i}")
                           for i in range(6)]
                    for k in range(KT):
                        hb = phb.tile([128, 512], BF16, tag="htb", name="hb")
                        nc.sync.dma_start(hb[:], ht[128 * k:128 * (k + 1),
                                                    SC * c:SC * (c + 1)])
                        first, last = k == 0, k == KT - 1
                        for h in range(NQH):
                            nc.tensor.matmul(ps6[h][:], wqb[:, 512 * k + 128 * h:512 * k + 128 * (h + 1)],
                                             hb[:], start=first, stop=last)
                        nc.tensor.matmul(ps6[4][:], wkb[:, 128 * k:128 * (k + 1)], hb[:],
                                         start=first, stop=last)
                        # V in natural [seq, dv] layout: hb-slice stationary,
                        # wvb moving -> no DMA transpose needed later.
                        # start=True clears has_written for the WHOLE bank, so
                        # only the very first sub-matmul may carry it; the
                        # other b4 regions auto-overwrite on first touch.
                        for b4 in range(4):
                            nc.tensor.matmul(ps6[5][:, 128 * b4:128 * (b4 + 1)],
                                             hb[:, 128 * b4:128 * (b4 + 1)],
                                             wvb[:, 128 * k:128 * (k + 1)],
                                             start=(first and b4 == 0), stop=last,
                                             skip_group_check=True)
                    # fast PSUM eviction (bf16): single wide copies free banks
                    # for the attention stage; rope then runs out of SBUF.
                    # q0/q1 first so pair-0 scores un-block earliest.
                    evs = [None] * 5
                    evs[0] = pev.tile([128, 512], BF16, tag="ev", name="ev0")
                    nc.scalar.copy(evs[0][:], ps6[0][:])
                    evs[1] = pev.tile([128, 512], BF16, tag="ev", name="ev1")
                    nc.scalar.copy(evs[1][:], ps6[1][:])
                    evs[2] = pev.tile([128, 512], BF16, tag="ev", name="ev2")
                    nc.vector.tensor_copy(evs[2][:], ps6[2][:])
                    evs[3] = pev.tile([128, 512], BF16, tag="ev", name="ev3")
                    nc.vector.tensor_copy(evs[3][:], ps6[3][:])
                    evs[4] = pev.tile([128, 512], BF16, tag="ev", name="ev4")
                    nc.scalar.copy(evs[4][:], ps6[4][:])
                    nc.scalar.copy(vnat[:, SC * c:SC * (c + 1)], ps6[5][:])
                _rope(qtb[0][:, SC * c:SC * (c + 1)], evs[0], c)
                _rope(qtb[1][:, SC * c:SC * (c + 1)], evs[1], c)
                _rope(ktb[:, SC * c:SC * (c + 1)], evs[4], c)
                while deferred:
                    deferred.pop(0)()
                return evs

            wo_ob = [None]

            def emit_wo_group(wc, wj, wn, watbs):
                """One Wo output tile [128q, 512hid] for chunk wc: 4 head-MMs,
                PSUM->SBUF copy (bf16). Groups of one query block share a wide
                ob tile; a single [128,4096] store fires on the last group."""
                po = pop.tile([128, 512], F32, tag="po", name="po")
                for h in range(NQH):
                    nc.tensor.matmul(po[:], watbs[h][:, 128 * wj:128 * (wj + 1)],
                                     wob[:, HID * h + 512 * wn:HID * h + 512 * (wn + 1)],
                                     start=(h == 0), stop=(h == NQH - 1))
                if wn == 0:
                    wo_ob[0] = posb.tile([128, HID], BF16, tag="osb", name="osb")
                ob = wo_ob[0]
                pcopy_rr(ob[:, 512 * wn:512 * (wn + 1)], po[:])
                if wn == 7:
                    # scalar queue: keeps wide stores off the ht-stream queue
                    nc.scalar.dma_start(out[SC * wc + 128 * wj:SC * wc + 128 * (wj + 1), :],
                                        ob[:])

            def attn_stage(c, prev, evs):
                # ---- attention for this chunk (past K/V only: sliding window),
                # with the PREVIOUS chunk's Wo matmul groups interleaved between
                # key-blocks so PE has independent work during rope/exp waits.
                wo_pending = []
                if prev is not None:
                    pc, patbs = prev
                    wo_pending = [(pc, j, n, patbs) for j in range(4) for n in range(8)]
                with tc.tile_pool(name="scps", bufs=2, space="PSUM") as psc, \
                     tc.tile_pool(name="pvps", bufs=2, space="PSUM") as ppv, \
                     tc.tile_pool(name="denps", bufs=2, space="PSUM") as pdn:
                    kbs = list(range(max(0, 4 * c - 8), 4 * c + 4))
                    first_kb, last_kb = kbs[0], kbs[-1]
                    # give PE independent work while DVE runs this chunk's rope
                    for _ in range(min(10, len(wo_pending))):
                        emit_wo_group(*wo_pending.pop(0))
                    atbs = []
                    for h0 in range(0, NQH, 2):
                        if h0 == 2:
                            # q2/q3 rope lands here so its DVE/DMA work queues
                            # behind pair-0's masks/copies, not ahead of them.
                            _rope(qtb[2][:, SC * c:SC * (c + 1)], evs[2], c)
                            _rope(qtb[3][:, SC * c:SC * (c + 1)], evs[3], c)
                        # process a PAIR of heads per key-block sweep: two
                        # independent score/exp chains per step keep ACT fed.
                        pvs = [ppv.tile([128, 512], F32, tag="pv", name="pv")
                               for _ in range(2)]
                        dens = [pdn.tile([1, 512], F32, tag="den", name="den")
                                for _ in range(2)]

                        def emit_pv(kb, pts):
                            # accumulate P@V and row-sums over exact visible slices.
                            sl = kb - 4 * c + 8
                            lo, hi = _slot_region(sl)
                            vsl = vnat[:, 128 * kb:128 * (kb + 1)]
                            for i in range(2):
                                nc.tensor.matmul(pvs[i][:, lo:hi], vsl, pts[i][:, lo:hi],
                                                 start=(kb == first_kb), stop=(kb == last_kb),
                                                 skip_group_check=True)
                                nc.tensor.matmul(dens[i][:, lo:hi], ones_t[:], pts[i][:, lo:hi],
                                                 start=(kb == first_kb), stop=(kb == last_kb),
                                                 skip_group_check=True)

                        prevkb = None
                        for kb in kbs:
                            sl = kb - 4 * c + 8
                            lo, hi = _slot_region(sl)
                            pts = []
                            for i in range(2):
                                sc = psc.tile([128, 512], F32, tag="sc", name="sc")
                                nc.tensor.matmul(sc[:, lo:hi], ktb[:, 128 * kb:128 * (kb + 1)],
                                                 qtb[h0 + i][:, SC * c + lo:SC * c + hi],
                                                 start=True, stop=True)
                                pt = ppt.tile([128, 512], BF16, tag="pt", name="pt")
                                nc.scalar.activation(pt[:, lo:hi], sc[:, lo:hi], EXP, scale=SCALE)
                                if sl <= 3:
                                    mofs = 128 * sl
                                    nc.gpsimd.tensor_tensor(pt[:, mofs:mofs + 128],
                                                            pt[:, mofs:mofs + 128], mw_t[:], MULT)
                                elif sl >= 8:
                                    mofs = 128 * (sl - 8)
                                    nc.gpsimd.tensor_tensor(pt[:, mofs:mofs + 128],
                                                            pt[:, mofs:mofs + 128], mc_t[:], MULT)
                                pts.append(pt)
                            if prevkb is not None:
                                emit_pv(*prevkb)
                            if wo_pending:
                                emit_wo_group(*wo_pending.pop(0))
                            prevkb = (kb, pts)
                        emit_pv(*prevkb)
                        if wo_pending:
                            emit_wo_group(*wo_pending.pop(0))
                        last_pair = h0 == 2
                        for i in range(2):
                            # evict pv/den out of PSUM promptly (frees banks for
                            # the next proj stage), then normalize from SBUF.
                            # The LAST pair's broadcast+multiply are deferred
                            # into the next proj stage: their results are only
                            # needed a chunk later, and running them here puts
                            # them ahead of the next chunk's rope in the
                            # GP/DVE FIFOs (priority inversion).
                            pvf = ppvf.tile([128, 512], F32, tag="pvf", name="pvf")
                            _copy(nc.scalar if i == 0 else nc.vector, pvf[:], pvs[i][:])
                            dre = pmisc.tile([1, 512], F32, tag="denr", name="denr")
                            nc.vector.reciprocal(dre[:], dens[i][:])
                            at = pat.tile([128, 512], BF16, tag="atb", name="atb")

                            def _norm(pvf=pvf, dre=dre, at=at):
                                dbc = pmisc.tile([128, 512], F32, tag="denb", name="denb")
                                nc.gpsimd.partition_broadcast(dbc[:], dre[:])
                                nc.vector.tensor_tensor(at[:], pvf[:], dbc[:], MULT)

                            if last_pair:
                                deferred.append(_norm)
                            else:
                                _norm()
                            atbs.append(at)
                    while wo_pending:
                        emit_wo_group(*wo_pending.pop(0))
                return atbs

            prev = None
            deferred = []
            for _it in range(niter):
                if fused:
                    for c in range(NCH):
                        evs = proj_stage(c)
                        prev = (c, attn_stage(c, prev, evs))
                else:
                    allevs = []
                    for c in range(NCH):
                        allevs.append(proj_stage(c))
                    for c in range(NCH):
                        prev = (c, attn_stage(c, prev, allevs[c]))
            # final chunk's Wo tail
            while deferred:
                deferred.pop(0)()
            pc, patbs = prev
            for j in range(4):
                for n in range(8):
                    emit_wo_group(pc, j, n, patbs)
            if "dbg_q0" in dr:
                nc.sync.dma_start(dr["dbg_q0"], qtb[0][:])
                nc.sync.dma_start(dr["dbg_k"], ktb[:])
                nc.sync.dma_start(dr["dbg_v"], vnat[:])
                for hh in range(4):
                    nc.sync.dma_start(dr[f"dbg_at{hh}"], patbs[hh][:])


_NC_CACHE = {}


def _build(niter=1, fused=True):
    import os
    fused = os.environ.get("KERNEL_FUSED", "1" if fused else "0") == "1"
    key = (niter, fused)
    if key in _NC_CACHE:
        return _NC_CACHE[key]
    nc = bacc.Bacc("TRN2", target_bir_lowering=False, debug=False,
                   enable_asserts=True, num_devices=NCORES)
    dr = {}

    def din(name, shape, dt=F32):
        dr[name] = nc.dram_tensor(name, shape, dt, kind="ExternalInput").ap()

    din("ht", [HID, S], BF16)
    din("wqt", [HID, NQH * D], BF16)
    din("wkt", [HID, D], BF16)
    din("wvt", [HID, D], BF16)
    din("wot", [NQH * D, HID], BF16)
    din("ctab", [128, S], BF16)
    din("stab", [128, S], BF16)
    din("mcaus", [128, 128], BF16)
    din("mwin", [128, 128], BF16)
    out = nc.dram_tensor("out", [S, HID], BF16, kind="ExternalOutput").ap()
    if os.environ.get("KERNEL_DEBUG") == "1":
        for nm, shp in [("dbg_q0", [128, S]), ("dbg_k", [128, S]), ("dbg_v", [128, S]),
                        ("dbg_at0", [128, 512]), ("dbg_at1", [128, 512]),
                        ("dbg_at2", [128, 512]), ("dbg_at3", [128, 512])]:
            dr[nm] = nc.dram_tensor(nm, shp, BF16, kind="ExternalOutput").ap()

    with tile.TileContext(nc) as tc:
        _program(tc, dr, out, niter, fused)
    nc.compile()
    _NC_CACHE[key] = nc
    return nc


def make_in_maps(inputs):
    hs = np.asarray(inputs["hidden_states"], dtype=np.float32)
    Wq = np.asarray(inputs["Wq"], dtype=np.float32)
    Wk = np.asarray(inputs["Wk"], dtype=np.float32)
    Wv = np.asarray(inputs["Wv"], dtype=np.float32)
    Wo = np.asarray(inputs["Wo"], dtype=np.float32)
    pos = np.asarray(inputs["position_ids"]).reshape(-1)

    assert hs.shape == (1, S, HID), hs.shape
    H = hs[0]
    BF = ml_dtypes.bfloat16
    HT = np.ascontiguousarray(H.T.astype(BF))

    # RoPE tables in [d%64, s] layout (bf16, mirroring the reference math)
    inv = (1.0 / (ROPE_BASE ** (np.arange(0, D, 2, dtype=np.float32) / D))).astype(np.float32)
    ang = pos.astype(np.float32)[None, :] * inv[:, None]          # [64, S]
    cos64 = np.cos(ang).astype(np.float32)
    sin64 = np.sin(ang).astype(np.float32)
    ctab = np.concatenate([cos64, cos64], axis=0).astype(BF)      # [128, S]
    stab = np.concatenate([-sin64, sin64], axis=0).astype(BF)     # sign-baked

    kk = np.arange(128)[:, None]
    qq = np.arange(128)[None, :]
    mcaus = (qq >= kk).astype(BF)   # causal diag block, [k,q]
    mwin = (qq < kk).astype(BF)     # window-edge block, [k,q]

    in_maps = []
    for i in range(NCORES):
        in_maps.append({
            "ht": HT,
            "wqt": np.ascontiguousarray(Wq[512 * i:512 * (i + 1), :].T.astype(BF)),
            "wkt": np.ascontiguousarray(Wk[128 * i:128 * (i + 1), :].T.astype(BF)),
            "wvt": np.ascontiguousarray(Wv[128 * i:128 * (i + 1), :].T.astype(BF)),
            "wot": np.ascontiguousarray(Wo[:, 512 * i:512 * (i + 1)].T.astype(BF)),
            "ctab": ctab,
            "stab": stab,
            "mcaus": mcaus,
            "mwin": mwin,
        })

    return in_maps


def kernel(**inputs):
    in_maps = make_in_maps(inputs)
    nc = _build()
    res = run_bass_kernel_spmd(nc, in_maps, core_ids=list(range(NCORES)))

    acc = np.zeros((S, HID), dtype=np.float64)
    for r in res.results:
        acc += r["out"].astype(np.float64)
    return acc.astype(np.float32).reshape(1, S, HID)
